# revision 28
# baseline (speedup 1.0000x reference)
"""Trainium2 Bass kernel for Exaone4-style GQA attention block (T=2048, HID=4096,
H=32 q-heads, HK=8 kv-heads, D=128, sliding window 1023, QK-RMSNorm + NeoX RoPE).

Sharding: tensor-parallel over heads across 8 NeuronCores. Core m owns q-heads
[4m, 4m+4) and kv-head m (GQA group-aligned), plus the matching o_proj column
slice; per-core partial outputs are summed on the host (the all-reduce).

Device design notes:
 - Attention is GQA-packed: one S^T matmul covers all 4 q-heads for a 128-query
   block (rhs columns = (head, t)), with the shared K-block / V-block as the
   stationary operand. All attention matmuls are uniform [128,128]x[128,512].
 - The per-(s-block) chain S -> mask -> exp -> PV/rowsum is emitted fine-grained
   and interleaved with qkv / o_proj projection matmuls so the PE never idles
   (keeps the HAM clock gate warm at 2.4 GHz).
 - ACT runs Exp only (one table load); QK-RMSNorm rsqrt is a Quake-style
   Newton iteration on DVE; column sums (softmax denominator, mean-square) use
   a [128,128] ones stationary operand so the result lands partition-replicated
   in PSUM - no partition broadcasts anywhere.
 - RoPE uses shared cos/sin tables plus per-partition norm-weight scalars via
   fused scalar_tensor_tensor ops; the d-half rotation is an SBUF-SBUF DMA.
 - All large matmuls use bf16 operands with fp32 PSUM accumulation.
"""

import sys

import numpy as np

if "/opt/trn_rl_repo" not in sys.path:
    sys.path.insert(0, "/opt/trn_rl_repo")

import ml_dtypes

BF16 = ml_dtypes.bfloat16

HID = 4096
H = 32
HK = 8
D = 128
WIN = 1023
THETA = 1000000.0
EPS = 1e-6
SCALE = D ** -0.5
M = 8            # cores
QH = H // M      # q heads per core (4)
NJ = QH + 2      # j-blocks in qkv^T output (4 q + 1 k + 1 v)
TB = 512         # t free-dim block
HB = 256         # half t block (x staging granularity)
NEG = -1.0e30
MAGIC = 0x5F3759DF

_PROG_CACHE = {}


def _build_program(T):
    """Build the (single-core SPMD) Bass program for sequence length T."""
    from contextlib import ExitStack

    import concourse.bass as bass  # noqa: F401
    import concourse.tile as tile
    from concourse import bacc, mybir
    from concourse.masks import make_identity

    f32 = mybir.dt.float32
    bf = mybir.dt.bfloat16
    i32 = mybir.dt.int32

    NT = T // TB          # 512-blocks (4)
    NU = T // 128         # 128-query blocks (16)
    NC = HID // 128       # contraction chunks (32)
    NOB = HID // 128      # o_proj output row blocks (32)

    mult = mybir.AluOpType.mult
    add = mybir.AluOpType.add
    sub = mybir.AluOpType.subtract
    shr = mybir.AluOpType.arith_shift_right
    Exp = mybir.ActivationFunctionType.Exp

    nc = bacc.Bacc(
        "TRN2",
        target_bir_lowering=False,
        debug=False,
        enable_asserts=False,
        num_devices=M,
    )

    # x pre-tiled on host: block (tb, half) = [128, (cq ci u)] fully contiguous
    xT_h = nc.dram_tensor(
        "xT", [NT * 2 * 128, (HID // 128) * HB], bf, kind="ExternalInput"
    )
    # qkv weights, j-major: [128, (j, c, f)]
    wT_h = nc.dram_tensor("wT", [128, NJ * NC * 128], bf, kind="ExternalInput")
    # o_proj weights, obp-major: [(obp, p), (jc, oi, o')]
    wo_h = nc.dram_tensor("woT2", [(NOB // 2) * 128, 4 * 256], bf, kind="ExternalInput")
    cwq_h = nc.dram_tensor("cwq", [128, T], bf, kind="ExternalInput")
    swq_h = nc.dram_tensor("swq", [128, T], bf, kind="ExternalInput")
    cwk_h = nc.dram_tensor("cwk", [128, T], bf, kind="ExternalInput")
    swk_h = nc.dram_tensor("swk", [128, T], bf, kind="ExternalInput")
    maskd_h = nc.dram_tensor("maskd", [128, 512], bf, kind="ExternalInput")
    maskw_h = nc.dram_tensor("maskw", [128, 512], bf, kind="ExternalInput")
    # out pre-tiled: block (tb, obp) = [128, 2*TB]
    outT_h = nc.dram_tensor(
        "outT", [NT * (HID // 256) * 128, 2 * TB], bf, kind="ExternalOutput"
    )

    wTr = wT_h.ap().rearrange("p (j c f) -> p j c f", j=NJ, c=NC)

    with tile.TileContext(nc) as tc, ExitStack() as ctx:
        consts = ctx.enter_context(tc.tile_pool(name="consts", bufs=1))
        persist = ctx.enter_context(tc.tile_pool(name="persist", bufs=1))
        xthp = ctx.enter_context(tc.tile_pool(name="xthp", bufs=3))
        wop = ctx.enter_context(tc.tile_pool(name="wop", bufs=3))
        stp = ctx.enter_context(tc.tile_pool(name="stp", bufs=4))
        sqp = ctx.enter_context(tc.tile_pool(name="sqp", bufs=2))
        qrp = ctx.enter_context(tc.tile_pool(name="qrp", bufs=2))
        rtp = ctx.enter_context(tc.tile_pool(name="rtp", bufs=3))
        ntp = ctx.enter_context(tc.tile_pool(name="ntp", bufs=2))
        y0p = ctx.enter_context(tc.tile_pool(name="y0p", bufs=2))
        sclp = ctx.enter_context(tc.tile_pool(name="sclp", bufs=2))
        esp = ctx.enter_context(tc.tile_pool(name="esp", bufs=4))
        rbp = ctx.enter_context(tc.tile_pool(name="rbp", bufs=2))
        atp = ctx.enter_context(tc.tile_pool(name="atp", bufs=2))
        osp = ctx.enter_context(tc.tile_pool(name="osp", bufs=2))
        # PSUM: 8 banks total: 3 (S) + 1 (pv) + 2 (ms/rs) + 2 (proj chains)
        spsum = ctx.enter_context(tc.tile_pool(name="spsum", bufs=3, space="PSUM"))
        pvps = ctx.enter_context(tc.tile_pool(name="pvps", bufs=1, space="PSUM"))
        smps = ctx.enter_context(tc.tile_pool(name="smps", bufs=2, space="PSUM"))
        prps = ctx.enter_context(tc.tile_pool(name="prps", bufs=2, space="PSUM"))

        # ---- resident constants (loads emitted below, interleaved) ----------
        w_sb = consts.tile([128, NJ, NC, 128], bf)
        cwq_sb = consts.tile([128, T], bf)
        swq_sb = consts.tile([128, T], bf)
        cwk_sb = consts.tile([128, T], bf)
        swk_sb = consts.tile([128, T], bf)
        maskd_sb = consts.tile([128, 512], bf)
        maskw_sb = consts.tile([128, 512], bf)
        ident = consts.tile([128, 128], bf)
        ones_bf = consts.tile([128, 128], bf)
        magic_i = consts.tile([128, TB], i32)
        one_i = consts.tile([128, TB], i32)

        # ---- persistent activations ----------------------------------------
        qT = persist.tile([128, NU, QH, 128], bf)   # roped+normed q^T
        kT = persist.tile([128, T], bf)             # roped+normed k^T
        Vt = persist.tile([128, NU, 128], bf)       # v in [s, d] layout

        attnTs = {}
        stages = {}
        xth = {}

        xTr = xT_h.ap().rearrange("(b p) u -> b p u", p=128)

        def emit_xth_load(tbn, h, nsplit=2):
            """Stage x for (tb, half): [128, cq, ci, HB]; split across DMA
            queues for transfer parallelism."""
            t = xthp.tile([128, 8, 4, HB], bf, tag="xth", name=f"xth_{tbn}_{h}")
            src = xTr[tbn * 2 + h].rearrange("p (cq ci u) -> p cq ci u", cq=8, ci=4)
            step = 8 // nsplit
            for g in range(nsplit):
                nc.sync.dma_start(
                    t[:, g * step : (g + 1) * step],
                    src[:, g * step : (g + 1) * step],
                )
            xth[(tbn, h)] = t

        def emit_qkv_chain(tbn, h, j):
            """Half-chain: qkv projection for j-block j, t columns [h*HB, h*HB+HB)."""
            ps = prps.tile([128, HB], f32, tag="proj", name=f"qkv_{tbn}_{h}_{j}")
            xt = xth[(tbn, h)]
            for cq in range(8):
                for ci in range(4):
                    c = cq * 4 + ci
                    nc.tensor.matmul(
                        ps,
                        lhsT=w_sb[:, j, c, :],
                        rhs=xt[:, cq, ci, :],
                        start=(c == 0),
                        stop=(c == NC - 1),
                    )
            if h == 0:
                stages[(tbn, j)] = stp.tile(
                    [128, TB], bf, tag="stage", name=f"st_{tbn}_{j}"
                )
            st = stages[(tbn, j)]
            nc.vector.tensor_copy(st[:, h * HB : (h + 1) * HB], ps)

        def emit_rms_rope(tbn, j):
            """RMS-normalize + RoPE j-block j of tb (j<QH: q head j; j==QH: k)."""
            t0 = tbn * TB
            ts_ = slice(t0, t0 + TB)
            st = stages.pop((tbn, j))
            # d-half rotation via SBUF->SBUF DMA (runs while rms computes)
            qr = qrp.tile([128, TB], bf, tag="qrot", name=f"qr_{tbn}_{j}")
            nc.gpsimd.dma_start(qr[0:64, :], st[64:128, :])
            nc.gpsimd.dma_start(qr[64:128, :], st[0:64, :])
            # mean-square via ones-matmul (partition-replicated result)
            sq = sqp.tile([128, TB], bf, tag="sq", name=f"sq_{tbn}_{j}")
            nc.vector.tensor_tensor(sq, st, st, mult)
            ms = smps.tile([128, TB], f32, tag="small", name=f"ms_{tbn}_{j}")
            nc.tensor.matmul(ms, lhsT=ones_bf, rhs=sq, start=True, stop=True)
            # rsqrt(ms) via magic-constant seed + 1 Newton step (all DVE).
            # sqrt(D) and the 1/sqrt(D) score scale are folded into the host
            # tables, so the raw column sum-of-squares is the right input.
            sh = ntp.tile([128, TB], i32, tag="nt", name=f"sh_{tbn}_{j}")
            nc.vector.tensor_tensor(sh, ms.bitcast(i32), one_i, shr)
            y0i = y0p.tile([128, TB], i32, tag="y0", name=f"y0_{tbn}_{j}")
            nc.vector.tensor_tensor(y0i, magic_i, sh, sub)
            y0 = y0i.bitcast(f32)
            a2 = ntp.tile([128, TB], f32, tag="nt", name=f"a2_{tbn}_{j}")
            nc.vector.tensor_tensor(a2, y0, y0, mult)
            d2 = ntp.tile([128, TB], f32, tag="nt", name=f"d2_{tbn}_{j}")
            nc.vector.scalar_tensor_tensor(d2, a2, -0.5, ms, mult, mult)
            scl = sclp.tile([128, TB], f32, tag="scl", name=f"scl_{tbn}_{j}")
            nc.vector.scalar_tensor_tensor(scl, d2, 1.5, y0, add, mult)
            # rope: dest = (st*cw + rot(st)*sw) * scl; the norm weights and
            # score scale are folded into the host tables. The two table
            # multiplies run on GpSimd (plain TT) to unload the DVE.
            cw, sw = (cwq_sb, swq_sb) if j < QH else (cwk_sb, swk_sb)
            a = rtp.tile([128, TB], f32, tag="rt", name=f"ra_{tbn}_{j}")
            nc.gpsimd.tensor_tensor(a, st, cw[:, ts_], mult)
            b = rtp.tile([128, TB], f32, tag="rt", name=f"rb_{tbn}_{j}")
            nc.gpsimd.tensor_tensor(b, qr, sw[:, ts_], mult)
            cc = rtp.tile([128, TB], f32, tag="rt", name=f"rc_{tbn}_{j}")
            nc.vector.tensor_tensor(cc, a, b, add)
            if j < QH:
                dest = qT[:, 4 * tbn : 4 * tbn + 4, j, :]
            else:
                dest = kT[:, ts_]
            nc.vector.tensor_tensor(dest, cc, scl, mult)

        def emit_vtrans(tbn):
            """v: transpose [d, t] -> [s, d] blocks via PE."""
            st = stages.pop((tbn, NJ - 1))
            for q in range(4):
                pst = prps.tile([128, 128], bf, tag="proj", name=f"vt_{tbn}_{q}")
                nc.tensor.transpose(pst, st[:, q * 128 : (q + 1) * 128], ident)
                nc.vector.tensor_copy(Vt[:, tbn * 4 + q, :], pst)

        def emit_attn(u):
            """Attention for query block u: all 4 heads packed per matmul."""
            first = max(0, u - 8)
            sbs = list(range(first, u + 1))
            tbn = u // 4
            ur = u % 4
            if ur == 0:
                attnTs[tbn] = atp.tile(
                    [128, QH, TB], bf, tag="attnT", name=f"attnT_{tbn}"
                )
            pv = pvps.tile([128, TB], f32, tag="pv", name=f"pv_{u}")
            rs = smps.tile([128, TB], f32, tag="small", name=f"rs_{u}")
            ess = {}
            for sb in sbs:
                ps = spsum.tile([128, TB], f32, tag="spsum", name=f"s_{u}_{sb}")
                nc.tensor.matmul(
                    ps,
                    lhsT=kT[:, sb * 128 : (sb + 1) * 128],
                    rhs=qT[:, u],
                    start=True,
                    stop=True,
                )
                # No mask before exp: windowed scores are O(1) so exp cannot
                # overflow; out-of-band entries are zeroed on the es tile
                # (GpSimd) which is exactly exp(score - inf). This keeps the
                # S -> exp -> PV chain off the DVE queue entirely.
                es = esp.tile([128, TB], bf, tag="es", name=f"es_{u}_{sb}")
                nc.scalar.activation(es, ps, Exp)
                if sb == u:
                    nc.gpsimd.tensor_tensor(es, es, maskd_sb, mult)
                elif u - sb == 8:
                    nc.gpsimd.tensor_tensor(es, es, maskw_sb, mult)
                ess[sb] = es
            # accumulate starting from an unmasked block so the group start
            # never waits on the mask multiply
            order = [sb for sb in sbs if sb != u and u - sb != 8]
            order += [sb for sb in sbs if sb == u or u - sb == 8]
            for i, sb in enumerate(order):
                es = ess[sb]
                last = i == len(order) - 1
                nc.tensor.matmul(
                    pv, lhsT=Vt[:, sb, :], rhs=es,
                    start=(i == 0), stop=last, skip_group_check=True,
                )
                nc.tensor.matmul(
                    rs, lhsT=ones_bf, rhs=es,
                    start=(i == 0), stop=last, skip_group_check=True,
                )
            rb = rbp.tile([128, TB], f32, tag="rbn", name=f"rbn_{u}")
            nc.vector.reciprocal_approx_fast(rb, rs)
            at = attnTs[tbn]
            nc.vector.tensor_tensor(
                at[:, :, ur * 128 : (ur + 1) * 128],
                pv.rearrange("p (h t) -> p h t", t=128),
                rb.rearrange("p (h t) -> p h t", t=128),
                mult,
            )

        worP = wo_h.ap().rearrange(
            "(b p) (jc oi o) -> p b jc oi o", p=128, jc=4, oi=2
        )
        outP = outT_h.ap().rearrange("(b p) (oi u) -> p b oi u", p=128, u=TB)

        def emit_oproj_pair(tbn, pp):
            """o_proj for (tb, obp-pair pp): wo streamed, one out DMA."""
            at = attnTs[tbn]
            wot = wop.tile([128, 2, 4, 2, 128], bf, tag="wo", name=f"wo_{tbn}_{pp}")
            nc.sync.dma_start(wot, worP[:, 2 * pp : 2 * pp + 2])
            ost = osp.tile([128, 2, 2, TB], bf, tag="ost", name=f"ost_{tbn}_{pp}")
            for bi in range(2):
                for oi in range(2):
                    ps = prps.tile(
                        [128, TB], f32, tag="proj", name=f"op_{tbn}_{pp}_{bi}_{oi}"
                    )
                    for jc in range(QH):
                        nc.tensor.matmul(
                            ps,
                            lhsT=wot[:, bi, jc, oi, :],
                            rhs=at[:, jc, :],
                            start=(jc == 0),
                            stop=(jc == QH - 1),
                        )
                    nc.vector.tensor_copy(ost[:, bi, oi, :], ps)
            base = tbn * (NOB // 2) + 2 * pp
            nc.gpsimd.dma_start(outP[:, base : base + 2], ost)

        # ---- prologue -------------------------------------------------------
        # Interleave the first x/w DMAs so matmuls start as soon as the first
        # half of x and the first j-block of w have landed.
        PROLOG_J = [4, 5, 0, 1, 2, 3]

        def emit_w_load(j):
            nc.sync.dma_start(w_sb[:, j, 0:16], wTr[:, j, 0:16])
            nc.sync.dma_start(w_sb[:, j, 16:32], wTr[:, j, 16:32])

        emit_w_load(4)
        emit_xth_load(0, 0, nsplit=4)
        emit_w_load(5)
        emit_xth_load(0, 1, nsplit=4)
        for j in (0, 1, 2, 3):
            emit_w_load(j)
        for t_, h_ in (
            (cwq_sb, cwq_h), (swq_sb, swq_h), (cwk_sb, cwk_h), (swk_sb, swk_h),
            (maskd_sb, maskd_h), (maskw_sb, maskw_h),
        ):
            nc.sync.dma_start(t_, h_.ap())
        make_identity(nc, ident)
        nc.vector.memset(ones_bf, 1.0)
        nc.gpsimd.memset(magic_i, MAGIC)
        nc.gpsimd.memset(one_i, 1)

        for j in PROLOG_J:
            emit_qkv_chain(0, 0, j)
            emit_qkv_chain(0, 1, j)
            if j < NJ - 1:
                emit_rms_rope(0, j)
            else:
                emit_vtrans(0)
        emit_xth_load(1, 0)
        emit_xth_load(1, 1)

        # chains of tb+1 emitted during tb over ur0..ur2 (k and v first so the
        # shared kT/Vt ropes land with maximal slack; ur3 stays chain-free so
        # the last q rope finishes well before tb+1's first S matmul)
        CHAIN_SCHED = [
            [(0, 4), (1, 4), (0, 5), (1, 5)],
            [(0, 0), (1, 0), (0, 1), (1, 1)],
            [(0, 2), (1, 2), (0, 3), (1, 3)],
            [],
        ]
        # o_proj pairs per ur: more at ur3 to fill the chain-free stretch
        OPROJ_SCHED = [(0, 1), (1, 3), (3, 5), (5, 8)]

        # ---- steady state ---------------------------------------------------
        for tbn in range(NT):
            for ur in range(4):
                u = 4 * tbn + ur
                emit_attn(u)
                if tbn >= 1:
                    for pp in range(*OPROJ_SCHED[ur]):
                        emit_oproj_pair(tbn - 1, pp)
                if tbn + 1 < NT:
                    for (h, j) in CHAIN_SCHED[ur]:
                        emit_qkv_chain(tbn + 1, h, j)
                        if h == 1:
                            if j < NJ - 1:
                                emit_rms_rope(tbn + 1, j)
                            else:
                                emit_vtrans(tbn + 1)
                if tbn + 2 < NT and ur == 3:
                    emit_xth_load(tbn + 2, 0)
                    emit_xth_load(tbn + 2, 1)
        # ---- epilogue -------------------------------------------------------
        for pp in range(NOB // 4):
            emit_oproj_pair(NT - 1, pp)

    nc.compile()
    return nc


def _get_program(T):
    if T not in _PROG_CACHE:
        _PROG_CACHE[T] = _build_program(T)
    return _PROG_CACHE[T]


def _host_prep(positions, hidden_states, wqkv, wo, q_norm_w, k_norm_w):
    """Build the 8 per-core input maps (host-side sharding + table prep)."""
    T = hidden_states.shape[0]
    pos = np.asarray(positions).astype(np.float64)
    hs = np.asarray(hidden_states, dtype=np.float32)
    wqkv = np.asarray(wqkv, dtype=np.float32)
    wo = np.asarray(wo, dtype=np.float32)
    qw = np.asarray(q_norm_w, dtype=np.float64)
    kw = np.asarray(k_norm_w, dtype=np.float64)

    half = D // 2
    inv_freq = 1.0 / (THETA ** (np.arange(0, D, 2, dtype=np.float64) / D))  # [64]
    th = pos[:, None] * inv_freq[None, :]          # [T, 64]
    cos = np.cos(th).T                             # [64, T]
    sin = np.sin(th).T

    # rope tables with norm weights folded in; q side folds SCALE*sqrt(D)=1,
    # k side folds sqrt(D) (the device rsqrt is of the raw sum of squares).
    sqD = float(np.sqrt(D))

    def tables(w, scale):
        cw = np.empty((D, T), np.float64)
        sw = np.empty((D, T), np.float64)
        cw[:half] = cos * (w[:half, None] * scale)
        cw[half:] = cos * (w[half:, None] * scale)
        sw[:half] = -sin * (w[half:, None] * scale)
        sw[half:] = sin * (w[:half, None] * scale)
        return cw.astype(BF16), sw.astype(BF16)

    cwq, swq = tables(qw, 1.0)
    cwk, swk = tables(kw, sqD)

    # 0/1 keep-masks over (s in 128, (h in 4) x (t in 128)), applied to es
    si = np.arange(128)[:, None]
    ti = np.arange(128)[None, :]
    md = np.where(ti >= si, 1.0, 0.0).astype(BF16)   # diag: keep t >= s
    mw = np.where(ti < si, 1.0, 0.0).astype(BF16)    # window edge: t < s
    maskd = np.tile(md, (1, 4))
    maskw = np.tile(mw, (1, 4))

    # x tiled: block (tb, half) = [128, (cq ci u)] contiguous
    NTb, NCq = T // TB, HID // 512
    xT = np.ascontiguousarray(
        hs.T.reshape(NCq, 4, 128, NTb, 2, HB)
        .transpose(3, 4, 2, 0, 1, 5)
        .reshape(NTb * 2 * 128, NCq * 4 * HB)
    ).astype(BF16)

    in_maps = []
    for m in range(M):
        wq_m = wqkv[m * QH * D : (m + 1) * QH * D]            # [512, HID]
        wk_m = wqkv[H * D + m * D : H * D + (m + 1) * D]      # [128, HID]
        wv_m = wqkv[(H + HK) * D + m * D : (H + HK) * D + (m + 1) * D]
        Wm = np.concatenate([wq_m, wk_m, wv_m], axis=0)       # [768, HID]
        # [p, j, c, f] layout
        wT = np.ascontiguousarray(
            Wm.T.reshape(HID // 128, 128, NJ, 128)
            .transpose(1, 2, 0, 3)
            .reshape(128, NJ * (HID // 128) * 128)
        ).astype(BF16)
        # o_proj: [obp, p, jc, oi, o'] -> [(obp p), (jc oi o')]
        wom = wo[:, m * QH * D : (m + 1) * QH * D]            # [HID, 512]
        woT2 = np.ascontiguousarray(
            wom.reshape(HID // 256, 2, 128, 4, 128)
            .transpose(0, 4, 3, 1, 2)
            .reshape((HID // 256) * 128, 4 * 256)
        ).astype(BF16)
        in_maps.append(
            {
                "xT": xT,
                "wT": wT,
                "woT2": woT2,
                "cwq": cwq,
                "swq": swq,
                "cwk": cwk,
                "swk": swk,
                "maskd": maskd,
                "maskw": maskw,
            }
        )
    return in_maps


def _run(in_maps, T, trace=False):
    from concourse import bass_utils

    nc = _get_program(T)
    res = bass_utils.run_bass_kernel_spmd(
        nc, in_maps, core_ids=list(range(M)), trace=trace
    )
    return res


def kernel(positions, hidden_states, wqkv, wo, q_norm_w, k_norm_w, _trace=False):
    T = hidden_states.shape[0]
    in_maps = _host_prep(positions, hidden_states, wqkv, wo, q_norm_w, k_norm_w)
    res = _run(in_maps, T, trace=_trace)
    NTb, NOBp = T // TB, HID // 256
    acc = np.zeros((NTb, NOBp, 128, 2, TB), np.float64)
    for r in res.results:
        acc += r["outT"].astype(np.float64).reshape(NTb, NOBp, 128, 2, TB)
    # untile: out[t, o] with o = (2*obp + oi)*128 + p, t = tb*TB + u
    out = np.ascontiguousarray(
        acc.transpose(0, 4, 1, 3, 2).reshape(T, HID)
    ).astype(np.float32)
    kernel._last_results = res
    return out


# revision 34
# speedup vs baseline: 1.0593x; 1.0593x over previous
"""Trainium2 Bass kernel for Exaone4-style GQA attention block (T=2048, HID=4096,
H=32 q-heads, HK=8 kv-heads, D=128, sliding window 1023, QK-RMSNorm + NeoX RoPE).

Sharding: tensor-parallel over heads across 8 NeuronCores. Core m owns q-heads
[4m, 4m+4) and kv-head m (GQA group-aligned), plus the matching o_proj column
slice; per-core partial outputs are summed on the host (the all-reduce).

Device design notes:
 - Attention is GQA-packed: one S^T matmul covers all 4 q-heads for a 128-query
   block (rhs columns = (head, t)), with the shared K-block / V-block as the
   stationary operand. All attention matmuls are uniform [128,128]x[128,512].
 - The per-(s-block) chain S -> mask -> exp -> PV/rowsum is emitted fine-grained
   and interleaved with qkv / o_proj projection matmuls so the PE never idles
   (keeps the HAM clock gate warm at 2.4 GHz).
 - ACT runs Exp only (one table load); QK-RMSNorm rsqrt is a Quake-style
   Newton iteration on DVE; column sums (softmax denominator, mean-square) use
   a [128,128] ones stationary operand so the result lands partition-replicated
   in PSUM - no partition broadcasts anywhere.
 - RoPE uses shared cos/sin tables plus per-partition norm-weight scalars via
   fused scalar_tensor_tensor ops; the d-half rotation is an SBUF-SBUF DMA.
 - All large matmuls use bf16 operands with fp32 PSUM accumulation.
"""

import sys

import numpy as np

if "/opt/trn_rl_repo" not in sys.path:
    sys.path.insert(0, "/opt/trn_rl_repo")

import ml_dtypes

BF16 = ml_dtypes.bfloat16

HID = 4096
H = 32
HK = 8
D = 128
WIN = 1023
THETA = 1000000.0
EPS = 1e-6
SCALE = D ** -0.5
M = 8            # cores
QH = H // M      # q heads per core (4)
NJ = QH + 2      # j-blocks in qkv^T output (4 q + 1 k + 1 v)
TB = 512         # t free-dim block
HB = 256         # half t block (x staging granularity)
NEG = -1.0e30
MAGIC = 0x5F3759DF

_PROG_CACHE = {}


def _build_program(T):
    """Build the (single-core SPMD) Bass program for sequence length T."""
    from contextlib import ExitStack

    import concourse.bass as bass  # noqa: F401
    import concourse.tile as tile
    from concourse import bacc, mybir
    from concourse.masks import make_identity

    f32 = mybir.dt.float32
    bf = mybir.dt.bfloat16
    i32 = mybir.dt.int32

    NT = T // TB          # 512-blocks (4)
    NU = T // 128         # 128-query blocks (16)
    NC = HID // 128       # contraction chunks (32)
    NOB = HID // 128      # o_proj output row blocks (32)

    mult = mybir.AluOpType.mult
    add = mybir.AluOpType.add
    sub = mybir.AluOpType.subtract
    shr = mybir.AluOpType.arith_shift_right
    Exp = mybir.ActivationFunctionType.Exp

    nc = bacc.Bacc(
        "TRN2",
        target_bir_lowering=False,
        debug=False,
        enable_asserts=False,
        num_devices=M,
    )

    # x pre-tiled on host: block (tb, half) = [128, (cq ci u)] fully contiguous
    xT_h = nc.dram_tensor(
        "xT", [NT * 2 * 128, (HID // 128) * HB], bf, kind="ExternalInput"
    )
    # qkv weights, j-major: [128, (j, c, f)]
    wT_h = nc.dram_tensor("wT", [128, NJ * NC * 128], bf, kind="ExternalInput")
    # o_proj weights, obp-major: [(obp, p), (jc, oi, o')]
    wo_h = nc.dram_tensor("woT2", [(NOB // 2) * 128, 4 * 256], bf, kind="ExternalInput")
    cwq_h = nc.dram_tensor("cwq", [128, T], bf, kind="ExternalInput")
    swq_h = nc.dram_tensor("swq", [128, T], bf, kind="ExternalInput")
    cwk_h = nc.dram_tensor("cwk", [128, T], bf, kind="ExternalInput")
    swk_h = nc.dram_tensor("swk", [128, T], bf, kind="ExternalInput")
    # triangular -inf mask factors: S-psum = maskT.T @ ident4 (+= scores)
    maskd_h = nc.dram_tensor("maskd", [128, 128], bf, kind="ExternalInput")
    maskw_h = nc.dram_tensor("maskw", [128, 128], bf, kind="ExternalInput")
    id4_h = nc.dram_tensor("ident4", [128, 512], bf, kind="ExternalInput")
    # out pre-tiled: block (tb, obp) = [128, 2*TB]
    outT_h = nc.dram_tensor(
        "outT", [NT * (HID // 256) * 128, 2 * TB], bf, kind="ExternalOutput"
    )

    wTr = wT_h.ap().rearrange("p (j c f) -> p j c f", j=NJ, c=NC)

    with tile.TileContext(nc) as tc, ExitStack() as ctx:
        consts = ctx.enter_context(tc.tile_pool(name="consts", bufs=1))
        persist = ctx.enter_context(tc.tile_pool(name="persist", bufs=1))
        xthp = ctx.enter_context(tc.tile_pool(name="xthp", bufs=3))
        wop = ctx.enter_context(tc.tile_pool(name="wop", bufs=3))
        stp = ctx.enter_context(tc.tile_pool(name="stp", bufs=4))
        sqp = ctx.enter_context(tc.tile_pool(name="sqp", bufs=2))
        qrp = ctx.enter_context(tc.tile_pool(name="qrp", bufs=2))
        rtp = ctx.enter_context(tc.tile_pool(name="rtp", bufs=3))
        ntp = ctx.enter_context(tc.tile_pool(name="ntp", bufs=2))
        y0p = ctx.enter_context(tc.tile_pool(name="y0p", bufs=2))
        sclp = ctx.enter_context(tc.tile_pool(name="sclp", bufs=2))
        esp = ctx.enter_context(tc.tile_pool(name="esp", bufs=4))
        rbp = ctx.enter_context(tc.tile_pool(name="rbp", bufs=2))
        atp = ctx.enter_context(tc.tile_pool(name="atp", bufs=2))
        osp = ctx.enter_context(tc.tile_pool(name="osp", bufs=2))
        # PSUM: 8 banks total: 3 (S) + 1 (pv) + 2 (ms/rs) + 2 (proj chains)
        spsum = ctx.enter_context(tc.tile_pool(name="spsum", bufs=3, space="PSUM"))
        pvps = ctx.enter_context(tc.tile_pool(name="pvps", bufs=1, space="PSUM"))
        smps = ctx.enter_context(tc.tile_pool(name="smps", bufs=2, space="PSUM"))
        prps = ctx.enter_context(tc.tile_pool(name="prps", bufs=2, space="PSUM"))

        # ---- resident constants (loads emitted below, interleaved) ----------
        w_sb = consts.tile([128, NJ, NC, 128], bf)
        cwq_sb = consts.tile([128, T], bf)
        swq_sb = consts.tile([128, T], bf)
        cwk_sb = consts.tile([128, T], bf)
        swk_sb = consts.tile([128, T], bf)
        maskd_sb = consts.tile([128, 128], bf)
        maskw_sb = consts.tile([128, 128], bf)
        id4_sb = consts.tile([128, 512], bf)
        ident = consts.tile([128, 128], bf)
        ones_bf = consts.tile([128, 128], bf)
        magic_i = consts.tile([128, TB], i32)
        one_i = consts.tile([128, TB], i32)

        # ---- persistent activations ----------------------------------------
        qT = persist.tile([128, NU, QH, 128], bf)   # roped+normed q^T
        kT = persist.tile([128, T], bf)             # roped+normed k^T
        Vt = persist.tile([128, NU, 128], bf)       # v in [s, d] layout

        attnTs = {}
        stages = {}
        xth = {}

        xTr = xT_h.ap().rearrange("(b p) u -> b p u", p=128)

        def emit_xth_load(tbn, h, nsplit=2):
            """Stage x for (tb, half): [128, cq, ci, HB]; split across DMA
            queues for transfer parallelism."""
            t = xthp.tile([128, 8, 4, HB], bf, tag="xth", name=f"xth_{tbn}_{h}")
            src = xTr[tbn * 2 + h].rearrange("p (cq ci u) -> p cq ci u", cq=8, ci=4)
            step = 8 // nsplit
            for g in range(nsplit):
                nc.sync.dma_start(
                    t[:, g * step : (g + 1) * step],
                    src[:, g * step : (g + 1) * step],
                )
            xth[(tbn, h)] = t

        def emit_qkv_chain(tbn, h, j):
            """Half-chain: qkv projection for j-block j, t columns [h*HB, h*HB+HB)."""
            ps = prps.tile([128, HB], f32, tag="proj", name=f"qkv_{tbn}_{h}_{j}")
            xt = xth[(tbn, h)]
            for cq in range(8):
                for ci in range(4):
                    c = cq * 4 + ci
                    nc.tensor.matmul(
                        ps,
                        lhsT=w_sb[:, j, c, :],
                        rhs=xt[:, cq, ci, :],
                        start=(c == 0),
                        stop=(c == NC - 1),
                    )
            if h == 0:
                stages[(tbn, j)] = stp.tile(
                    [128, TB], bf, tag="stage", name=f"st_{tbn}_{j}"
                )
            st = stages[(tbn, j)]
            nc.vector.tensor_copy(st[:, h * HB : (h + 1) * HB], ps)

        def emit_rms_rope(tbn, j):
            """RMS-normalize + RoPE j-block j of tb (j<QH: q head j; j==QH: k)."""
            t0 = tbn * TB
            ts_ = slice(t0, t0 + TB)
            st = stages.pop((tbn, j))
            # d-half rotation via SBUF->SBUF DMA (runs while rms computes)
            qr = qrp.tile([128, TB], bf, tag="qrot", name=f"qr_{tbn}_{j}")
            nc.gpsimd.dma_start(qr[0:64, :], st[64:128, :])
            nc.gpsimd.dma_start(qr[64:128, :], st[0:64, :])
            # mean-square via ones-matmul (partition-replicated result)
            sq = sqp.tile([128, TB], bf, tag="sq", name=f"sq_{tbn}_{j}")
            nc.vector.tensor_tensor(sq, st, st, mult)
            ms = smps.tile([128, TB], f32, tag="small", name=f"ms_{tbn}_{j}")
            nc.tensor.matmul(ms, lhsT=ones_bf, rhs=sq, start=True, stop=True)
            # rsqrt(ms) via magic-constant seed + 1 Newton step (all DVE).
            # sqrt(D) and the 1/sqrt(D) score scale are folded into the host
            # tables, so the raw column sum-of-squares is the right input.
            sh = ntp.tile([128, TB], i32, tag="nt", name=f"sh_{tbn}_{j}")
            nc.vector.tensor_tensor(sh, ms.bitcast(i32), one_i, shr)
            y0i = y0p.tile([128, TB], i32, tag="y0", name=f"y0_{tbn}_{j}")
            nc.vector.tensor_tensor(y0i, magic_i, sh, sub)
            y0 = y0i.bitcast(f32)
            a2 = ntp.tile([128, TB], f32, tag="nt", name=f"a2_{tbn}_{j}")
            nc.vector.tensor_tensor(a2, y0, y0, mult)
            d2 = ntp.tile([128, TB], f32, tag="nt", name=f"d2_{tbn}_{j}")
            nc.vector.scalar_tensor_tensor(d2, a2, -0.5, ms, mult, mult)
            scl = sclp.tile([128, TB], f32, tag="scl", name=f"scl_{tbn}_{j}")
            nc.vector.scalar_tensor_tensor(scl, d2, 1.5, y0, add, mult)
            # rope: dest = (st*cw + rot(st)*sw) * scl; the norm weights and
            # score scale are folded into the host tables. The two table
            # multiplies run on GpSimd (plain TT) to unload the DVE.
            cw, sw = (cwq_sb, swq_sb) if j < QH else (cwk_sb, swk_sb)
            a = rtp.tile([128, TB], f32, tag="rt", name=f"ra_{tbn}_{j}")
            nc.gpsimd.tensor_tensor(a, st, cw[:, ts_], mult)
            b = rtp.tile([128, TB], f32, tag="rt", name=f"rb_{tbn}_{j}")
            nc.gpsimd.tensor_tensor(b, qr, sw[:, ts_], mult)
            cc = rtp.tile([128, TB], f32, tag="rt", name=f"rc_{tbn}_{j}")
            nc.vector.tensor_tensor(cc, a, b, add)
            if j < QH:
                dest = qT[:, 4 * tbn : 4 * tbn + 4, j, :]
            else:
                dest = kT[:, ts_]
            nc.vector.tensor_tensor(dest, cc, scl, mult)

        def emit_vtrans(tbn):
            """v: transpose [d, t] -> [s, d] blocks via PE."""
            st = stages.pop((tbn, NJ - 1))
            for q in range(4):
                pst = prps.tile([128, 128], bf, tag="proj", name=f"vt_{tbn}_{q}")
                nc.tensor.transpose(pst, st[:, q * 128 : (q + 1) * 128], ident)
                nc.vector.tensor_copy(Vt[:, tbn * 4 + q, :], pst)

        def emit_attn(u):
            """Attention for query block u: all 4 heads packed per matmul."""
            first = max(0, u - 8)
            sbs = list(range(first, u + 1))
            tbn = u // 4
            ur = u % 4
            if ur == 0:
                attnTs[tbn] = atp.tile(
                    [128, QH, TB], bf, tag="attnT", name=f"attnT_{tbn}"
                )
            pv = pvps.tile([128, TB], f32, tag="pv", name=f"pv_{u}")
            rs = smps.tile([128, TB], f32, tag="small", name=f"rs_{u}")
            for i, sb in enumerate(sbs):
                ps = spsum.tile([128, TB], f32, tag="spsum", name=f"s_{u}_{sb}")
                # masked blocks: pre-write the -inf triangle into the psum via
                # a mask-matmul, then accumulate the scores onto it. Keeps the
                # whole S -> exp -> PV chain on PE+ACT only.
                masked = sb == u or u - sb == 8
                if masked:
                    nc.tensor.matmul(
                        ps,
                        lhsT=maskd_sb if sb == u else maskw_sb,
                        rhs=id4_sb,
                        start=True,
                        stop=False,
                    )
                nc.tensor.matmul(
                    ps,
                    lhsT=kT[:, sb * 128 : (sb + 1) * 128],
                    rhs=qT[:, u],
                    start=not masked,
                    stop=True,
                )
                es = esp.tile([128, TB], bf, tag="es", name=f"es_{u}_{sb}")
                nc.scalar.activation(es, ps, Exp)
                last = i == len(sbs) - 1
                nc.tensor.matmul(
                    pv, lhsT=Vt[:, sb, :], rhs=es,
                    start=(i == 0), stop=last, skip_group_check=True,
                )
                nc.tensor.matmul(
                    rs, lhsT=ones_bf, rhs=es,
                    start=(i == 0), stop=last, skip_group_check=True,
                )
            rb = rbp.tile([128, TB], f32, tag="rbn", name=f"rbn_{u}")
            nc.vector.reciprocal_approx_fast(rb, rs)
            at = attnTs[tbn]
            nc.vector.tensor_tensor(
                at[:, :, ur * 128 : (ur + 1) * 128],
                pv.rearrange("p (h t) -> p h t", t=128),
                rb.rearrange("p (h t) -> p h t", t=128),
                mult,
            )

        worP = wo_h.ap().rearrange(
            "(b p) (jc oi o) -> p b jc oi o", p=128, jc=4, oi=2
        )
        outP = outT_h.ap().rearrange("(b p) (oi u) -> p b oi u", p=128, u=TB)

        def emit_oproj_pair(tbn, pp):
            """o_proj for (tb, obp-pair pp): wo streamed, one out DMA."""
            at = attnTs[tbn]
            wot = wop.tile([128, 2, 4, 2, 128], bf, tag="wo", name=f"wo_{tbn}_{pp}")
            nc.sync.dma_start(wot, worP[:, 2 * pp : 2 * pp + 2])
            ost = osp.tile([128, 2, 2, TB], bf, tag="ost", name=f"ost_{tbn}_{pp}")
            for bi in range(2):
                for oi in range(2):
                    ps = prps.tile(
                        [128, TB], f32, tag="proj", name=f"op_{tbn}_{pp}_{bi}_{oi}"
                    )
                    for jc in range(QH):
                        nc.tensor.matmul(
                            ps,
                            lhsT=wot[:, bi, jc, oi, :],
                            rhs=at[:, jc, :],
                            start=(jc == 0),
                            stop=(jc == QH - 1),
                        )
                    nc.vector.tensor_copy(ost[:, bi, oi, :], ps)
            base = tbn * (NOB // 2) + 2 * pp
            nc.gpsimd.dma_start(outP[:, base : base + 2], ost)

        # ---- prologue -------------------------------------------------------
        # Interleave the first x/w DMAs so matmuls start as soon as the first
        # half of x and the first j-block of w have landed.
        PROLOG_J = [4, 5, 0, 1, 2, 3]

        def emit_w_load(j):
            nc.sync.dma_start(w_sb[:, j, 0:16], wTr[:, j, 0:16])
            nc.sync.dma_start(w_sb[:, j, 16:32], wTr[:, j, 16:32])

        emit_w_load(4)
        emit_xth_load(0, 0, nsplit=4)
        emit_w_load(5)
        emit_xth_load(0, 1, nsplit=4)
        for j in (0, 1, 2, 3):
            emit_w_load(j)
        for t_, h_ in (
            (cwq_sb, cwq_h), (swq_sb, swq_h), (cwk_sb, cwk_h), (swk_sb, swk_h),
            (maskd_sb, maskd_h), (maskw_sb, maskw_h), (id4_sb, id4_h),
        ):
            nc.sync.dma_start(t_, h_.ap())
        make_identity(nc, ident)
        nc.vector.memset(ones_bf, 1.0)
        nc.gpsimd.memset(magic_i, MAGIC)
        nc.gpsimd.memset(one_i, 1)

        for j in PROLOG_J:
            emit_qkv_chain(0, 0, j)
            emit_qkv_chain(0, 1, j)
            if j < NJ - 1:
                emit_rms_rope(0, j)
            else:
                emit_vtrans(0)
        emit_xth_load(1, 0)
        emit_xth_load(1, 1)

        # chains of tb+1 emitted during tb over ur0..ur2 (k and v first so the
        # shared kT/Vt ropes land with maximal slack; ur3 stays chain-free so
        # the last q rope finishes well before tb+1's first S matmul)
        CHAIN_SCHED = [
            [(0, 4), (1, 4), (0, 5), (1, 5)],
            [(0, 0), (1, 0), (0, 1), (1, 1)],
            [(0, 2), (1, 2), (0, 3), (1, 3)],
            [],
        ]
        # o_proj pairs per ur: more at ur3 to fill the chain-free stretch
        OPROJ_SCHED = [(0, 1), (1, 3), (3, 5), (5, 8)]

        # ---- steady state ---------------------------------------------------
        for tbn in range(NT):
            for ur in range(4):
                u = 4 * tbn + ur
                emit_attn(u)
                if tbn >= 1:
                    for pp in range(*OPROJ_SCHED[ur]):
                        emit_oproj_pair(tbn - 1, pp)
                if tbn + 1 < NT:
                    for (h, j) in CHAIN_SCHED[ur]:
                        emit_qkv_chain(tbn + 1, h, j)
                        if h == 1:
                            if j < NJ - 1:
                                emit_rms_rope(tbn + 1, j)
                            else:
                                emit_vtrans(tbn + 1)
                if tbn + 2 < NT and ur == 3:
                    emit_xth_load(tbn + 2, 0)
                    emit_xth_load(tbn + 2, 1)
        # ---- epilogue -------------------------------------------------------
        for pp in range(NOB // 4):
            emit_oproj_pair(NT - 1, pp)

    nc.compile()
    return nc


def _get_program(T):
    if T not in _PROG_CACHE:
        _PROG_CACHE[T] = _build_program(T)
    return _PROG_CACHE[T]


def _host_prep(positions, hidden_states, wqkv, wo, q_norm_w, k_norm_w):
    """Build the 8 per-core input maps (host-side sharding + table prep)."""
    T = hidden_states.shape[0]
    pos = np.asarray(positions).astype(np.float64)
    hs = np.asarray(hidden_states, dtype=np.float32)
    wqkv = np.asarray(wqkv, dtype=np.float32)
    wo = np.asarray(wo, dtype=np.float32)
    qw = np.asarray(q_norm_w, dtype=np.float64)
    kw = np.asarray(k_norm_w, dtype=np.float64)

    half = D // 2
    inv_freq = 1.0 / (THETA ** (np.arange(0, D, 2, dtype=np.float64) / D))  # [64]
    th = pos[:, None] * inv_freq[None, :]          # [T, 64]
    cos = np.cos(th).T                             # [64, T]
    sin = np.sin(th).T

    # rope tables with norm weights folded in; q side folds SCALE*sqrt(D)=1,
    # k side folds sqrt(D) (the device rsqrt is of the raw sum of squares).
    sqD = float(np.sqrt(D))

    def tables(w, scale):
        cw = np.empty((D, T), np.float64)
        sw = np.empty((D, T), np.float64)
        cw[:half] = cos * (w[:half, None] * scale)
        cw[half:] = cos * (w[half:, None] * scale)
        sw[:half] = -sin * (w[half:, None] * scale)
        sw[half:] = sin * (w[:half, None] * scale)
        return cw.astype(BF16), sw.astype(BF16)

    cwq, swq = tables(qw, 1.0)
    cwk, swk = tables(kw, sqD)

    # triangular mask factors [k, s]: psum mask = maskT.T @ ident4 gives
    # mask[s, (h,t)] = maskT[t mod 128, s]
    ki = np.arange(128)[:, None]
    si = np.arange(128)[None, :]
    maskd = np.where(ki < si, NEG, 0.0).astype(BF16)   # kill t < s on diag
    maskw = np.where(ki >= si, NEG, 0.0).astype(BF16)  # kill t >= s on edge
    id4 = np.tile(np.eye(128, dtype=np.float32), (1, 4)).astype(BF16)

    # x tiled: block (tb, half) = [128, (cq ci u)] contiguous
    NTb, NCq = T // TB, HID // 512
    xT = np.ascontiguousarray(
        hs.T.reshape(NCq, 4, 128, NTb, 2, HB)
        .transpose(3, 4, 2, 0, 1, 5)
        .reshape(NTb * 2 * 128, NCq * 4 * HB)
    ).astype(BF16)

    in_maps = []
    for m in range(M):
        wq_m = wqkv[m * QH * D : (m + 1) * QH * D]            # [512, HID]
        wk_m = wqkv[H * D + m * D : H * D + (m + 1) * D]      # [128, HID]
        wv_m = wqkv[(H + HK) * D + m * D : (H + HK) * D + (m + 1) * D]
        Wm = np.concatenate([wq_m, wk_m, wv_m], axis=0)       # [768, HID]
        # [p, j, c, f] layout
        wT = np.ascontiguousarray(
            Wm.T.reshape(HID // 128, 128, NJ, 128)
            .transpose(1, 2, 0, 3)
            .reshape(128, NJ * (HID // 128) * 128)
        ).astype(BF16)
        # o_proj: [obp, p, jc, oi, o'] -> [(obp p), (jc oi o')]
        wom = wo[:, m * QH * D : (m + 1) * QH * D]            # [HID, 512]
        woT2 = np.ascontiguousarray(
            wom.reshape(HID // 256, 2, 128, 4, 128)
            .transpose(0, 4, 3, 1, 2)
            .reshape((HID // 256) * 128, 4 * 256)
        ).astype(BF16)
        in_maps.append(
            {
                "xT": xT,
                "wT": wT,
                "woT2": woT2,
                "cwq": cwq,
                "swq": swq,
                "cwk": cwk,
                "swk": swk,
                "maskd": maskd,
                "maskw": maskw,
                "ident4": id4,
            }
        )
    return in_maps


def _run(in_maps, T, trace=False):
    from concourse import bass_utils

    nc = _get_program(T)
    res = bass_utils.run_bass_kernel_spmd(
        nc, in_maps, core_ids=list(range(M)), trace=trace
    )
    return res


def kernel(positions, hidden_states, wqkv, wo, q_norm_w, k_norm_w, _trace=False):
    T = hidden_states.shape[0]
    in_maps = _host_prep(positions, hidden_states, wqkv, wo, q_norm_w, k_norm_w)
    res = _run(in_maps, T, trace=_trace)
    NTb, NOBp = T // TB, HID // 256
    acc = np.zeros((NTb, NOBp, 128, 2, TB), np.float64)
    for r in res.results:
        acc += r["outT"].astype(np.float64).reshape(NTb, NOBp, 128, 2, TB)
    # untile: out[t, o] with o = (2*obp + oi)*128 + p, t = tb*TB + u
    out = np.ascontiguousarray(
        acc.transpose(0, 4, 1, 3, 2).reshape(T, HID)
    ).astype(np.float32)
    kernel._last_results = res
    return out


# revision 36
# speedup vs baseline: 1.0940x; 1.0328x over previous
"""Trainium2 Bass kernel for Exaone4-style GQA attention block (T=2048, HID=4096,
H=32 q-heads, HK=8 kv-heads, D=128, sliding window 1023, QK-RMSNorm + NeoX RoPE).

Sharding: tensor-parallel over heads across 8 NeuronCores. Core m owns q-heads
[4m, 4m+4) and kv-head m (GQA group-aligned), plus the matching o_proj column
slice; per-core partial outputs are summed on the host (the all-reduce).

Device design notes:
 - Attention is GQA-packed: one S^T matmul covers all 4 q-heads for a 128-query
   block (rhs columns = (head, t)), with the shared K-block / V-block as the
   stationary operand. All attention matmuls are uniform [128,128]x[128,512].
 - The per-(s-block) chain S -> mask -> exp -> PV/rowsum is emitted fine-grained
   and interleaved with qkv / o_proj projection matmuls so the PE never idles
   (keeps the HAM clock gate warm at 2.4 GHz).
 - ACT runs Exp only (one table load); QK-RMSNorm rsqrt is a Quake-style
   Newton iteration on DVE; column sums (softmax denominator, mean-square) use
   a [128,128] ones stationary operand so the result lands partition-replicated
   in PSUM - no partition broadcasts anywhere.
 - RoPE uses shared cos/sin tables plus per-partition norm-weight scalars via
   fused scalar_tensor_tensor ops; the d-half rotation is an SBUF-SBUF DMA.
 - All large matmuls use bf16 operands with fp32 PSUM accumulation.
"""

import sys

import numpy as np

if "/opt/trn_rl_repo" not in sys.path:
    sys.path.insert(0, "/opt/trn_rl_repo")

import ml_dtypes

BF16 = ml_dtypes.bfloat16

HID = 4096
H = 32
HK = 8
D = 128
WIN = 1023
THETA = 1000000.0
EPS = 1e-6
SCALE = D ** -0.5
M = 8            # cores
QH = H // M      # q heads per core (4)
NJ = QH + 2      # j-blocks in qkv^T output (4 q + 1 k + 1 v)
TB = 512         # t free-dim block
HB = 256         # half t block (x staging granularity)
NEG = -1.0e30
MAGIC = 0x5F3759DF

_PROG_CACHE = {}


def _build_program(T):
    """Build the (single-core SPMD) Bass program for sequence length T."""
    from contextlib import ExitStack

    import concourse.bass as bass  # noqa: F401
    import concourse.tile as tile
    from concourse import bacc, mybir
    from concourse.masks import make_identity

    f32 = mybir.dt.float32
    bf = mybir.dt.bfloat16
    i32 = mybir.dt.int32

    NT = T // TB          # 512-blocks (4)
    NU = T // 128         # 128-query blocks (16)
    NC = HID // 128       # contraction chunks (32)
    NOB = HID // 128      # o_proj output row blocks (32)

    mult = mybir.AluOpType.mult
    add = mybir.AluOpType.add
    sub = mybir.AluOpType.subtract
    shr = mybir.AluOpType.arith_shift_right
    Exp = mybir.ActivationFunctionType.Exp

    nc = bacc.Bacc(
        "TRN2",
        target_bir_lowering=False,
        debug=False,
        enable_asserts=False,
        num_devices=M,
    )

    # x pre-tiled on host: block (tb, half) = [128, (cq ci u)] fully contiguous
    xT_h = nc.dram_tensor(
        "xT", [NT * 2 * 128, (HID // 128) * HB], bf, kind="ExternalInput"
    )
    # qkv weights, j-major: [128, (j, c, f)]
    wT_h = nc.dram_tensor("wT", [128, NJ * NC * 128], bf, kind="ExternalInput")
    # o_proj weights, obp-major: [(obp, p), (jc, oi, o')]
    wo_h = nc.dram_tensor("woT2", [(NOB // 2) * 128, 4 * 256], bf, kind="ExternalInput")
    cwq_h = nc.dram_tensor("cwq", [128, T], bf, kind="ExternalInput")
    swq_h = nc.dram_tensor("swq", [128, T], bf, kind="ExternalInput")
    cwk_h = nc.dram_tensor("cwk", [128, T], bf, kind="ExternalInput")
    swk_h = nc.dram_tensor("swk", [128, T], bf, kind="ExternalInput")
    # triangular -inf mask factors: S-psum = maskT.T @ ident4 (+= scores)
    maskd_h = nc.dram_tensor("maskd", [128, 128], bf, kind="ExternalInput")
    maskw_h = nc.dram_tensor("maskw", [128, 128], bf, kind="ExternalInput")
    id4_h = nc.dram_tensor("ident4", [128, 512], bf, kind="ExternalInput")
    # out pre-tiled: block (tb, obp) = [128, 2*TB]
    outT_h = nc.dram_tensor(
        "outT", [NT * (HID // 256) * 128, 2 * TB], bf, kind="ExternalOutput"
    )

    wTr = wT_h.ap().rearrange("p (j c f) -> p j c f", j=NJ, c=NC)

    with tile.TileContext(nc) as tc, ExitStack() as ctx:
        consts = ctx.enter_context(tc.tile_pool(name="consts", bufs=1))
        persist = ctx.enter_context(tc.tile_pool(name="persist", bufs=1))
        xthp = ctx.enter_context(tc.tile_pool(name="xthp", bufs=3))
        wop = ctx.enter_context(tc.tile_pool(name="wop", bufs=3))
        stp = ctx.enter_context(tc.tile_pool(name="stp", bufs=4))
        sqp = ctx.enter_context(tc.tile_pool(name="sqp", bufs=2))
        qrp = ctx.enter_context(tc.tile_pool(name="qrp", bufs=2))
        rtp = ctx.enter_context(tc.tile_pool(name="rtp", bufs=3))
        ntp = ctx.enter_context(tc.tile_pool(name="ntp", bufs=2))
        y0p = ctx.enter_context(tc.tile_pool(name="y0p", bufs=2))
        sclp = ctx.enter_context(tc.tile_pool(name="sclp", bufs=2))
        esp = ctx.enter_context(tc.tile_pool(name="esp", bufs=4))
        rbp = ctx.enter_context(tc.tile_pool(name="rbp", bufs=2))
        atp = ctx.enter_context(tc.tile_pool(name="atp", bufs=2))
        osp = ctx.enter_context(tc.tile_pool(name="osp", bufs=2))
        # PSUM: 8 banks total: 3 (S) + 1 (pv) + 2 (ms/rs) + 2 (proj chains)
        spsum = ctx.enter_context(tc.tile_pool(name="spsum", bufs=3, space="PSUM"))
        pvps = ctx.enter_context(tc.tile_pool(name="pvps", bufs=1, space="PSUM"))
        smps = ctx.enter_context(tc.tile_pool(name="smps", bufs=2, space="PSUM"))
        prps = ctx.enter_context(tc.tile_pool(name="prps", bufs=2, space="PSUM"))

        # ---- resident constants (loads emitted below, interleaved) ----------
        w_sb = consts.tile([128, NJ, NC, 128], bf)
        cwq_sb = consts.tile([128, T], bf)
        swq_sb = consts.tile([128, T], bf)
        cwk_sb = consts.tile([128, T], bf)
        swk_sb = consts.tile([128, T], bf)
        maskd_sb = consts.tile([128, 128], bf)
        maskw_sb = consts.tile([128, 128], bf)
        id4_sb = consts.tile([128, 512], bf)
        ident = consts.tile([128, 128], bf)
        ones_bf = consts.tile([128, 128], bf)
        magic_i = consts.tile([128, TB], i32)
        one_i = consts.tile([128, TB], i32)

        # ---- persistent activations ----------------------------------------
        qT = persist.tile([128, NU, QH, 128], bf)   # roped+normed q^T
        kT = persist.tile([128, T], bf)             # roped+normed k^T
        Vt = persist.tile([128, NU, 128], bf)       # v in [s, d] layout

        attnTs = {}
        stages = {}
        xth = {}

        xTr = xT_h.ap().rearrange("(b p) u -> b p u", p=128)

        def emit_xth_load(tbn, h, nsplit=2):
            """Stage x for (tb, half): [128, cq, ci, HB]; split across DMA
            queues for transfer parallelism."""
            t = xthp.tile([128, 8, 4, HB], bf, tag="xth", name=f"xth_{tbn}_{h}")
            src = xTr[tbn * 2 + h].rearrange("p (cq ci u) -> p cq ci u", cq=8, ci=4)
            step = 8 // nsplit
            for g in range(nsplit):
                nc.sync.dma_start(
                    t[:, g * step : (g + 1) * step],
                    src[:, g * step : (g + 1) * step],
                )
            xth[(tbn, h)] = t

        def emit_qkv_chain(tbn, h, j):
            """Half-chain: qkv projection for j-block j, t columns [h*HB, h*HB+HB)."""
            ps = prps.tile([128, HB], f32, tag="proj", name=f"qkv_{tbn}_{h}_{j}")
            xt = xth[(tbn, h)]
            for cq in range(8):
                for ci in range(4):
                    c = cq * 4 + ci
                    nc.tensor.matmul(
                        ps,
                        lhsT=w_sb[:, j, c, :],
                        rhs=xt[:, cq, ci, :],
                        start=(c == 0),
                        stop=(c == NC - 1),
                    )
            if h == 0:
                stages[(tbn, j)] = stp.tile(
                    [128, TB], bf, tag="stage", name=f"st_{tbn}_{j}"
                )
            st = stages[(tbn, j)]
            nc.vector.tensor_copy(st[:, h * HB : (h + 1) * HB], ps)

        def emit_rms_rope(tbn, j):
            """RMS-normalize + RoPE j-block j of tb (j<QH: q head j; j==QH: k)."""
            t0 = tbn * TB
            ts_ = slice(t0, t0 + TB)
            st = stages.pop((tbn, j))
            # d-half rotation via SBUF->SBUF DMA (runs while rms computes)
            qr = qrp.tile([128, TB], bf, tag="qrot", name=f"qr_{tbn}_{j}")
            nc.gpsimd.dma_start(qr[0:64, :], st[64:128, :])
            nc.gpsimd.dma_start(qr[64:128, :], st[0:64, :])
            # mean-square via ones-matmul (partition-replicated result)
            sq = sqp.tile([128, TB], bf, tag="sq", name=f"sq_{tbn}_{j}")
            nc.vector.tensor_tensor(sq, st, st, mult)
            ms = smps.tile([128, TB], f32, tag="small", name=f"ms_{tbn}_{j}")
            nc.tensor.matmul(ms, lhsT=ones_bf, rhs=sq, start=True, stop=True)
            # rsqrt(ms) via magic-constant seed + 1 Newton step (all DVE).
            # sqrt(D) and the 1/sqrt(D) score scale are folded into the host
            # tables, so the raw column sum-of-squares is the right input.
            sh = ntp.tile([128, TB], i32, tag="nt", name=f"sh_{tbn}_{j}")
            nc.vector.tensor_tensor(sh, ms.bitcast(i32), one_i, shr)
            y0i = y0p.tile([128, TB], i32, tag="y0", name=f"y0_{tbn}_{j}")
            nc.vector.tensor_tensor(y0i, magic_i, sh, sub)
            y0 = y0i.bitcast(f32)
            a2 = ntp.tile([128, TB], f32, tag="nt", name=f"a2_{tbn}_{j}")
            nc.vector.tensor_tensor(a2, y0, y0, mult)
            d2 = ntp.tile([128, TB], f32, tag="nt", name=f"d2_{tbn}_{j}")
            nc.vector.scalar_tensor_tensor(d2, a2, -0.5, ms, mult, mult)
            scl = sclp.tile([128, TB], f32, tag="scl", name=f"scl_{tbn}_{j}")
            nc.vector.scalar_tensor_tensor(scl, d2, 1.5, y0, add, mult)
            # rope: dest = (st*cw + rot(st)*sw) * scl; the norm weights and
            # score scale are folded into the host tables. The two table
            # multiplies run on GpSimd (plain TT) to unload the DVE.
            cw, sw = (cwq_sb, swq_sb) if j < QH else (cwk_sb, swk_sb)
            a = rtp.tile([128, TB], f32, tag="rt", name=f"ra_{tbn}_{j}")
            nc.gpsimd.tensor_tensor(a, st, cw[:, ts_], mult)
            b = rtp.tile([128, TB], f32, tag="rt", name=f"rb_{tbn}_{j}")
            nc.gpsimd.tensor_tensor(b, qr, sw[:, ts_], mult)
            cc = rtp.tile([128, TB], f32, tag="rt", name=f"rc_{tbn}_{j}")
            nc.vector.tensor_tensor(cc, a, b, add)
            if j < QH:
                dest = qT[:, 4 * tbn : 4 * tbn + 4, j, :]
            else:
                dest = kT[:, ts_]
            nc.vector.tensor_tensor(dest, cc, scl, mult)

        def emit_vtrans(tbn):
            """v: transpose [d, t] -> [s, d] blocks via PE."""
            st = stages.pop((tbn, NJ - 1))
            for q in range(4):
                pst = prps.tile([128, 128], bf, tag="proj", name=f"vt_{tbn}_{q}")
                nc.tensor.transpose(pst, st[:, q * 128 : (q + 1) * 128], ident)
                nc.vector.tensor_copy(Vt[:, tbn * 4 + q, :], pst)

        def emit_attn(u):
            """Attention for query block u: all 4 heads packed per matmul."""
            first = max(0, u - 8)
            sbs = list(range(first, u + 1))
            tbn = u // 4
            ur = u % 4
            if ur == 0:
                attnTs[tbn] = atp.tile(
                    [128, QH, TB], bf, tag="attnT", name=f"attnT_{tbn}"
                )
            pv = pvps.tile([128, TB], f32, tag="pv", name=f"pv_{u}")
            rs = smps.tile([128, TB], f32, tag="small", name=f"rs_{u}")
            for i, sb in enumerate(sbs):
                ps = spsum.tile([128, TB], f32, tag="spsum", name=f"s_{u}_{sb}")
                # masked blocks: pre-write the -inf triangle into the psum via
                # a mask-matmul, then accumulate the scores onto it. Keeps the
                # whole S -> exp -> PV chain on PE+ACT only.
                masked = sb == u or u - sb == 8
                if masked:
                    nc.tensor.matmul(
                        ps,
                        lhsT=maskd_sb if sb == u else maskw_sb,
                        rhs=id4_sb,
                        start=True,
                        stop=False,
                    )
                nc.tensor.matmul(
                    ps,
                    lhsT=kT[:, sb * 128 : (sb + 1) * 128],
                    rhs=qT[:, u],
                    start=not masked,
                    stop=True,
                )
                es = esp.tile([128, TB], bf, tag="es", name=f"es_{u}_{sb}")
                nc.scalar.activation(es, ps, Exp)
                last = i == len(sbs) - 1
                nc.tensor.matmul(
                    pv, lhsT=Vt[:, sb, :], rhs=es,
                    start=(i == 0), stop=last, skip_group_check=True,
                )
                nc.tensor.matmul(
                    rs, lhsT=ones_bf, rhs=es,
                    start=(i == 0), stop=last, skip_group_check=True,
                )
            rb = rbp.tile([128, TB], f32, tag="rbn", name=f"rbn_{u}")
            nc.vector.reciprocal_approx_fast(rb, rs)
            at = attnTs[tbn]
            nc.vector.tensor_tensor(
                at[:, :, ur * 128 : (ur + 1) * 128],
                pv.rearrange("p (h t) -> p h t", t=128),
                rb.rearrange("p (h t) -> p h t", t=128),
                mult,
            )

        worP = wo_h.ap().rearrange(
            "(b p) (jc oi o) -> p b jc oi o", p=128, jc=4, oi=2
        )
        outP = outT_h.ap().rearrange("(b p) (oi u) -> p b oi u", p=128, u=TB)

        def emit_oproj_pair(tbn, pp):
            """o_proj for (tb, obp-pair pp): wo streamed, one out DMA."""
            at = attnTs[tbn]
            wot = wop.tile([128, 2, 4, 2, 128], bf, tag="wo", name=f"wo_{tbn}_{pp}")
            nc.sync.dma_start(wot, worP[:, 2 * pp : 2 * pp + 2])
            ost = osp.tile([128, 2, 2, TB], bf, tag="ost", name=f"ost_{tbn}_{pp}")
            for bi in range(2):
                for oi in range(2):
                    ps = prps.tile(
                        [128, TB], f32, tag="proj", name=f"op_{tbn}_{pp}_{bi}_{oi}"
                    )
                    for jc in range(QH):
                        nc.tensor.matmul(
                            ps,
                            lhsT=wot[:, bi, jc, oi, :],
                            rhs=at[:, jc, :],
                            start=(jc == 0),
                            stop=(jc == QH - 1),
                        )
                    nc.vector.tensor_copy(ost[:, bi, oi, :], ps)
            base = tbn * (NOB // 2) + 2 * pp
            nc.gpsimd.dma_start(outP[:, base : base + 2], ost)

        # ---- prologue -------------------------------------------------------
        # Interleave the first x/w DMAs so matmuls start as soon as the first
        # half of x and the first j-block of w have landed.
        # k first (its rope gates every S of the next tb), v LAST: v has no
        # rope, so the final q-rope's serial DVE tail overlaps v's chain MMs
        PROLOG_J = [4, 0, 1, 2, 3, 5]

        def emit_w_load(j):
            nc.sync.dma_start(w_sb[:, j, 0:16], wTr[:, j, 0:16])
            nc.sync.dma_start(w_sb[:, j, 16:32], wTr[:, j, 16:32])

        emit_w_load(4)
        emit_xth_load(0, 0, nsplit=4)
        emit_w_load(5)
        emit_xth_load(0, 1, nsplit=4)
        for j in (0, 1, 2, 3):
            emit_w_load(j)
        for t_, h_ in (
            (cwq_sb, cwq_h), (swq_sb, swq_h), (cwk_sb, cwk_h), (swk_sb, swk_h),
            (maskd_sb, maskd_h), (maskw_sb, maskw_h), (id4_sb, id4_h),
        ):
            nc.sync.dma_start(t_, h_.ap())
        make_identity(nc, ident)
        nc.vector.memset(ones_bf, 1.0)
        nc.gpsimd.memset(magic_i, MAGIC)
        nc.gpsimd.memset(one_i, 1)

        for j in PROLOG_J:
            emit_qkv_chain(0, 0, j)
            emit_qkv_chain(0, 1, j)
            if j < NJ - 1:
                emit_rms_rope(0, j)
            else:
                emit_vtrans(0)
        emit_xth_load(1, 0)
        emit_xth_load(1, 1)

        # chains of tb+1 emitted during tb over ur0..ur2 (k and v first so the
        # shared kT/Vt ropes land with maximal slack; ur3 stays chain-free so
        # the last q rope finishes well before tb+1's first S matmul)
        CHAIN_SCHED = [
            [(0, 4), (1, 4), (0, 0), (1, 0)],
            [(0, 1), (1, 1), (0, 2), (1, 2)],
            [(0, 3), (1, 3), (0, 5), (1, 5)],
            [],
        ]
        # o_proj pairs per ur: more at ur3 to fill the chain-free stretch
        OPROJ_SCHED = [(0, 1), (1, 3), (3, 5), (5, 8)]

        # ---- steady state ---------------------------------------------------
        for tbn in range(NT):
            for ur in range(4):
                u = 4 * tbn + ur
                emit_attn(u)
                if tbn >= 1:
                    for pp in range(*OPROJ_SCHED[ur]):
                        emit_oproj_pair(tbn - 1, pp)
                if tbn + 1 < NT:
                    for (h, j) in CHAIN_SCHED[ur]:
                        emit_qkv_chain(tbn + 1, h, j)
                        if h == 1:
                            if j < NJ - 1:
                                emit_rms_rope(tbn + 1, j)
                            else:
                                emit_vtrans(tbn + 1)
                if tbn + 2 < NT and ur == 3:
                    emit_xth_load(tbn + 2, 0)
                    emit_xth_load(tbn + 2, 1)
        # ---- epilogue -------------------------------------------------------
        for pp in range(NOB // 4):
            emit_oproj_pair(NT - 1, pp)

    nc.compile()
    return nc


def _get_program(T):
    if T not in _PROG_CACHE:
        _PROG_CACHE[T] = _build_program(T)
    return _PROG_CACHE[T]


def _host_prep(positions, hidden_states, wqkv, wo, q_norm_w, k_norm_w):
    """Build the 8 per-core input maps (host-side sharding + table prep)."""
    T = hidden_states.shape[0]
    pos = np.asarray(positions).astype(np.float64)
    hs = np.asarray(hidden_states, dtype=np.float32)
    wqkv = np.asarray(wqkv, dtype=np.float32)
    wo = np.asarray(wo, dtype=np.float32)
    qw = np.asarray(q_norm_w, dtype=np.float64)
    kw = np.asarray(k_norm_w, dtype=np.float64)

    half = D // 2
    inv_freq = 1.0 / (THETA ** (np.arange(0, D, 2, dtype=np.float64) / D))  # [64]
    th = pos[:, None] * inv_freq[None, :]          # [T, 64]
    cos = np.cos(th).T                             # [64, T]
    sin = np.sin(th).T

    # rope tables with norm weights folded in; q side folds SCALE*sqrt(D)=1,
    # k side folds sqrt(D) (the device rsqrt is of the raw sum of squares).
    sqD = float(np.sqrt(D))

    def tables(w, scale):
        cw = np.empty((D, T), np.float64)
        sw = np.empty((D, T), np.float64)
        cw[:half] = cos * (w[:half, None] * scale)
        cw[half:] = cos * (w[half:, None] * scale)
        sw[:half] = -sin * (w[half:, None] * scale)
        sw[half:] = sin * (w[:half, None] * scale)
        return cw.astype(BF16), sw.astype(BF16)

    cwq, swq = tables(qw, 1.0)
    cwk, swk = tables(kw, sqD)

    # triangular mask factors [k, s]: psum mask = maskT.T @ ident4 gives
    # mask[s, (h,t)] = maskT[t mod 128, s]
    ki = np.arange(128)[:, None]
    si = np.arange(128)[None, :]
    maskd = np.where(ki < si, NEG, 0.0).astype(BF16)   # kill t < s on diag
    maskw = np.where(ki >= si, NEG, 0.0).astype(BF16)  # kill t >= s on edge
    id4 = np.tile(np.eye(128, dtype=np.float32), (1, 4)).astype(BF16)

    # x tiled: block (tb, half) = [128, (cq ci u)] contiguous
    NTb, NCq = T // TB, HID // 512
    xT = np.ascontiguousarray(
        hs.T.reshape(NCq, 4, 128, NTb, 2, HB)
        .transpose(3, 4, 2, 0, 1, 5)
        .reshape(NTb * 2 * 128, NCq * 4 * HB)
    ).astype(BF16)

    in_maps = []
    for m in range(M):
        wq_m = wqkv[m * QH * D : (m + 1) * QH * D]            # [512, HID]
        wk_m = wqkv[H * D + m * D : H * D + (m + 1) * D]      # [128, HID]
        wv_m = wqkv[(H + HK) * D + m * D : (H + HK) * D + (m + 1) * D]
        Wm = np.concatenate([wq_m, wk_m, wv_m], axis=0)       # [768, HID]
        # [p, j, c, f] layout
        wT = np.ascontiguousarray(
            Wm.T.reshape(HID // 128, 128, NJ, 128)
            .transpose(1, 2, 0, 3)
            .reshape(128, NJ * (HID // 128) * 128)
        ).astype(BF16)
        # o_proj: [obp, p, jc, oi, o'] -> [(obp p), (jc oi o')]
        wom = wo[:, m * QH * D : (m + 1) * QH * D]            # [HID, 512]
        woT2 = np.ascontiguousarray(
            wom.reshape(HID // 256, 2, 128, 4, 128)
            .transpose(0, 4, 3, 1, 2)
            .reshape((HID // 256) * 128, 4 * 256)
        ).astype(BF16)
        in_maps.append(
            {
                "xT": xT,
                "wT": wT,
                "woT2": woT2,
                "cwq": cwq,
                "swq": swq,
                "cwk": cwk,
                "swk": swk,
                "maskd": maskd,
                "maskw": maskw,
                "ident4": id4,
            }
        )
    return in_maps


def _run(in_maps, T, trace=False):
    from concourse import bass_utils

    nc = _get_program(T)
    res = bass_utils.run_bass_kernel_spmd(
        nc, in_maps, core_ids=list(range(M)), trace=trace
    )
    return res


def kernel(positions, hidden_states, wqkv, wo, q_norm_w, k_norm_w, _trace=False):
    T = hidden_states.shape[0]
    in_maps = _host_prep(positions, hidden_states, wqkv, wo, q_norm_w, k_norm_w)
    res = _run(in_maps, T, trace=_trace)
    NTb, NOBp = T // TB, HID // 256
    acc = np.zeros((NTb, NOBp, 128, 2, TB), np.float64)
    for r in res.results:
        acc += r["outT"].astype(np.float64).reshape(NTb, NOBp, 128, 2, TB)
    # untile: out[t, o] with o = (2*obp + oi)*128 + p, t = tb*TB + u
    out = np.ascontiguousarray(
        acc.transpose(0, 4, 1, 3, 2).reshape(T, HID)
    ).astype(np.float32)
    kernel._last_results = res
    return out
